# revision 2
# baseline (speedup 1.0000x reference)
"""DGMNet forward kernel for Trainium2, 8-core data parallel.

Strategy: shard the batch across 8 cores; activations live feature-major
([HID, nt] in SBUF) so every matmul streams batch along the free dim
with the small weights stationary. All matmul operands and elementwise
intermediates are fp16 (e5m10): matmuls stream at 1 column/cycle with
automatic fast-weight-load, DVE tensor ops run in 2x packed mode, and
end-to-end quantization error stays ~2.4e-3 relative (8x inside the
2e-2 gate). Biases fold into the matmuls via a ones row appended to xt.
Per step and 512-column tile: PE runs 8 matmuls (xt-part K=102 + S-part
K=128 per gate, PSUM-accumulated), Act one wide tanh over Z|G|R, DVE
handles the recurrence-critical ops (S*R, (G-1)*H, S update) while the
off-critical Z*S product goes to GPSIMD. The final projection row is
copied from PSUM by DVE and DMA'd out; Wf_b is added on the host.
"""

import warnings

warnings.filterwarnings("ignore")

import numpy as np

B = 262144
XD = 100
HID = 128
N_CORES = 8
BS = B // N_CORES  # 32768 rows per core
KXT = XD + 2  # x features + t row + ones row
NT = 512  # batch columns per tile

F16 = np.float16


def _build(n_steps: int, bs: int, nt: int, reps: int = 1, hw_loop: int = 0):
    import concourse.bacc as bacc
    import concourse.tile as tile
    import concourse.mybir as mybir
    from contextlib import ExitStack, nullcontext

    f32 = mybir.dt.float32
    f16 = mybir.dt.float16
    Tanh = mybir.ActivationFunctionType.Tanh
    sub = mybir.AluOpType.subtract
    mult = mybir.AluOpType.mult

    ntiles = bs // nt
    nc = bacc.Bacc("TRN2", target_bir_lowering=False, debug=False,
                   num_devices=N_CORES)

    xb = nc.dram_tensor("xb", [KXT, bs], f16, kind="ExternalInput").ap()
    w1b = nc.dram_tensor("w1b", [KXT, 5 * HID], f16,
                         kind="ExternalInput").ap()
    w2 = nc.dram_tensor("w2", [HID, 4 * HID], f16, kind="ExternalInput").ap()
    wf = nc.dram_tensor("wf", [HID, 1], f16, kind="ExternalInput").ap()
    out = nc.dram_tensor("out", [1, bs], f32, kind="ExternalOutput").ap()

    with tile.TileContext(nc) as tc:
        with ExitStack() as ctx:
            consts = ctx.enter_context(tc.tile_pool(name="consts", bufs=1))
            xpool = ctx.enter_context(tc.tile_pool(name="xp", bufs=7))
            spool = ctx.enter_context(tc.tile_pool(name="sp", bufs=12))
            zpool = ctx.enter_context(tc.tile_pool(name="zp", bufs=4))
            epool = ctx.enter_context(tc.tile_pool(name="ep", bufs=6))
            psum = ctx.enter_context(
                tc.tile_pool(name="ps", bufs=2, space="PSUM"))

            w1b_t = consts.tile([KXT, 5 * HID], f16)
            nc.sync.dma_start(w1b_t[:], w1b[:, :])
            w2_t = consts.tile([HID, 4 * HID], f16)
            nc.sync.dma_start(w2_t[:], w2[:, :])
            wf_t = consts.tile([HID, 1], f16)
            nc.sync.dma_start(wf_t[:], wf[:, :])

            def w1s(k):
                return w1b_t[:, k * HID:(k + 1) * HID]

            def w2s(k):
                return w2_t[:, k * HID:(k + 1) * HID]

            # Optional HW loop repeating the whole pass (timing rig only)
            loop_cm = (tc.For_i(0, hw_loop, 1,
                                hint_engines=(mybir.EngineType.PE,
                                              mybir.EngineType.Activation,
                                              mybir.EngineType.DVE,
                                              mybir.EngineType.SP,
                                              mybir.EngineType.Pool))
                       if hw_loop else nullcontext())
            ctx.enter_context(loop_cm)

            GRP = 6  # tiles software-pipelined per step loop
            SUB = 3  # S1 tanh grouping (3 PSUM banks per wide tanh)
            for j0 in range(0, ntiles * reps, GRP):
                js = [jj % ntiles for jj in
                      range(j0, min(j0 + GRP, ntiles * reps))]
                xr = {}
                S = {}
                for j in js:
                    xb_t = xpool.tile([KXT, nt], f16, tag="x", name="xb_t")
                    nc.sync.dma_start(xb_t[:], xb[:, j * nt:(j + 1) * nt])
                    xr[j] = xb_t[:]
                # grouped S1: SUB matmuls into one multi-bank PSUM tile,
                # a single wide tanh per sub-group
                for s0 in range(0, len(js), SUB):
                    sub_js = js[s0:s0 + SUB]
                    g = len(sub_js)
                    ps3 = psum.tile([HID, g * nt], f32, tag="zgr", name="ps3")
                    for i, j in enumerate(sub_js):
                        nc.tensor.matmul(ps3[:, i * nt:(i + 1) * nt], w1s(0),
                                         xr[j], start=True, stop=True)
                    SG = zpool.tile([HID, g * nt], f16, tag="SG", name="SG")
                    nc.scalar.activation(SG[:], ps3[:], Tanh)
                    for i, j in enumerate(sub_js):
                        S[j] = SG[:, i * nt:(i + 1) * nt]

                for _ in range(n_steps):
                    for j in js:
                        Sr = S[j][:]
                        pzgr = psum.tile([HID, 3 * nt], f32, tag="zgr",
                                         name="pzgr")
                        for k in range(3):
                            sl = pzgr[:, k * nt:(k + 1) * nt]
                            nc.tensor.matmul(sl, w1s(1 + k), xr[j],
                                             start=True, stop=False)
                            nc.tensor.matmul(sl, w2s(k), Sr,
                                             start=False, stop=True)
                        ZGR = zpool.tile([HID, 3 * nt], f16, tag="ZGR",
                                         name="ZGR")
                        nc.scalar.activation(ZGR[:], pzgr[:], Tanh)
                        Z = ZGR[:, 0:nt]
                        G = ZGR[:, nt:2 * nt]
                        R = ZGR[:, 2 * nt:3 * nt]

                        SR = epool.tile([HID, nt], f16, tag="SR", name="SR")
                        nc.vector.tensor_mul(SR[:], S[j][:], R)

                        ph = psum.tile([HID, nt], f32, tag="h", name="ph")
                        nc.tensor.matmul(ph[:], w1s(4), xr[j],
                                         start=True, stop=False)
                        nc.tensor.matmul(ph[:], w2s(3), SR[:],
                                         start=False, stop=True)

                        ZS = epool.tile([HID, nt], f16, tag="ZS", name="ZS")
                        nc.gpsimd.tensor_mul(ZS[:], Z, S[j][:])
                        T1 = epool.tile([HID, nt], f16, tag="T1", name="T1")
                        nc.vector.scalar_tensor_tensor(T1[:], G, 1.0, ph[:],
                                                       op0=sub, op1=mult)
                        Snew = spool.tile([HID, nt], f16, tag="S",
                                          name="Snew")
                        nc.vector.tensor_sub(Snew[:], ZS[:], T1[:])
                        S[j] = Snew

                for j in js:
                    po = psum.tile([1, nt], f32, tag="h", name="po")
                    nc.tensor.matmul(po[:], wf_t[:], S[j][:],
                                     start=True, stop=True)
                    o_t = xpool.tile([1, nt], f32, tag="o", name="o_t")
                    nc.vector.tensor_copy(o_t[:], po[:])
                    nc.sync.dma_start(out[:, j * nt:(j + 1) * nt], o_t[:])

    nc.compile()
    return nc


_cache = {}


def _get_nc(n_steps: int, bs: int = BS, nt: int = NT):
    key = (n_steps, bs, nt)
    if key not in _cache:
        _cache[key] = _build(n_steps, bs, nt)
    return _cache[key]


def _pack_host(x, t, Sw_w, Sw_b, Uz_w, Uz_b, Wsz_w, Wsz_b, Ug_w, Ug_b, Wsg_w,
               Wsg_b, Ur_w, Ur_b, Wsr_w, Wsr_b, Uh_w, Uh_b, Wsh_w, Wsh_b,
               Wf_w):
    f32 = np.float32
    b_total = x.shape[0]
    xt_full = np.empty((KXT, b_total), dtype=f32)
    xt_full[:XD, :] = np.asarray(x, dtype=f32).T
    xt_full[XD, :] = np.asarray(t, dtype=f32)[:, 0]
    xt_full[XD + 1, :] = 1.0
    xt_bf = xt_full.astype(F16)

    def blk(w, b):
        # [101, 128] weights + folded bias row -> [102, 128]
        return np.concatenate(
            [np.asarray(w, f32),
             np.asarray(b, f32).reshape(1, HID)], axis=0)

    w1b = np.concatenate([
        blk(Sw_w, Sw_b),
        blk(Uz_w, np.asarray(Uz_b) + np.asarray(Wsz_b)),
        blk(Ug_w, np.asarray(Ug_b) + np.asarray(Wsg_b)),
        blk(Ur_w, np.asarray(Ur_b) + np.asarray(Wsr_b)),
        blk(Uh_w, np.asarray(Uh_b) + np.asarray(Wsh_b)),
    ], axis=1).astype(F16)
    w2 = np.concatenate([Wsz_w, Wsg_w, Wsr_w, Wsh_w], axis=1).astype(F16)
    wf = np.asarray(Wf_w, dtype=np.float32).reshape(HID, 1).astype(F16)
    return xt_bf, w1b, w2, wf


def _in_maps(xt_bf, w1b, w2, wf, bs):
    maps = []
    for c in range(N_CORES):
        maps.append({
            "xb": np.ascontiguousarray(xt_bf[:, c * bs:(c + 1) * bs]),
            "w1b": w1b,
            "w2": w2,
            "wf": wf,
        })
    return maps


def kernel(x, t, Sw_w, Sw_b, Uz_w, Uz_b, Wsz_w, Wsz_b, Ug_w, Ug_b, Wsg_w,
           Wsg_b, Ur_w, Ur_b, Wsr_w, Wsr_b, Uh_w, Uh_b, Wsh_w, Wsh_b, Wf_w,
           Wf_b, n_layers):
    from concourse.bass_utils import run_bass_kernel_spmd

    x = np.asarray(x)
    t = np.asarray(t)
    b_total = x.shape[0]
    assert b_total % N_CORES == 0
    bs = b_total // N_CORES
    n_steps = int(n_layers) - 1

    packed = _pack_host(
        x, t, Sw_w, Sw_b, Uz_w, Uz_b, Wsz_w, Wsz_b, Ug_w, Ug_b, Wsg_w, Wsg_b,
        Ur_w, Ur_b, Wsr_w, Wsr_b, Uh_w, Uh_b, Wsh_w, Wsh_b, Wf_w)

    nc = _get_nc(n_steps, bs)
    in_maps = _in_maps(*packed, bs)

    res = None
    for attempt in range(3):
        try:
            res = run_bass_kernel_spmd(nc, in_maps,
                                       core_ids=list(range(N_CORES)))
            break
        except Exception:
            if attempt == 2:
                raise
            import time as _time
            _time.sleep(5.0)
    out = np.empty((b_total, 1), dtype=np.float32)
    bf = np.float32(np.asarray(Wf_b).reshape(-1)[0])
    for c in range(N_CORES):
        out[c * bs:(c + 1) * bs, 0] = res.results[c]["out"][0] + bf
    return out
